# revision 11
# baseline (speedup 1.0000x reference)
"""Trainium2 Bass kernel for MoE-with-LoRA-experts (nn_MoE_64098091925598).

Reference computation (N=8192 tokens, D=1024, E=8 experts, R=16, top-2):
    logits  = x @ W_gate.T                      [N, E]
    combine = scatter(softmax(top2(logits)))    [N, E] (2 nonzeros/row)
    moe     = sum_e combine[:,e] * (x @ A_e @ B_e)
    out     = moe + x @ W_base.T + b_base

Strategy: data-parallel over tokens across 8 NeuronCores (1024 tokens
per core); every core computes all 8 LoRA experts densely and masks by
the combine weights (H[n,(e,r)] = combine[n,e] * (x@A)[n,(e,r)], so the
weighted expert sum is one dense K=128 matmul H @ B_flat accumulated
into the same PSUM tile as the base linear).

v4 perf structure (vs v1's 76us):
- Loads are HBM-bound (~300 GB/s aggregate no matter how many DGE
  rings), so in-bytes are minimized: W_base/A/B are bf16 (host-cast),
  output is stored bf16 and upcast on host. x stays f32 so the top-2
  selection matches the fp32 reference; matmuls against bf16 weights
  use mixed-dtype operands (f32r x side, bf16 weight side) so no
  on-device conversion is needed.
- Loads run ONLY on the sync+gpsimd rings, in consumption order with
  contiguous host-prepped tiles. The Scalar engine issues no loads:
  its queue would head-of-line block on DGE ring backpressure and
  starve the sigmoid/combine chain (v2/v3 lesson). Scalar does the
  sigmoids and the output stores.
- HAM: the PE clock is throttled to 1.2 GHz until ~3.4us of sustained
  high-duty matmul activity. 512-wide garbage matmuls run before and
  between the DMA-paced gating matmuls to un-throttle early.
- Per 512-token half: gating (8 wide matmuls + 4 PE transposes), lora
  up-proj, dt0 base-accumulation c-streamed against W_base arrivals,
  combine-transpose + rank-expand, H@B into the same psum banks, then
  dt1 with everything resident. The DVE combine chain overlaps the
  base c-loop so the PE never waits on it.
"""

import numpy as np
import ml_dtypes

import concourse.mybir as mybir
import concourse.tile as tile
from concourse import bacc
from concourse.bass_utils import run_bass_kernel_spmd
from concourse.masks import make_identity

N_TOK, D, E, R, TOPK = 8192, 1024, 8, 16, 2
CORES = 8
NS = N_TOK // CORES  # tokens per core
ER = E * R  # 128, stacked expert-rank dim
DC = D // 128  # 8 contraction chunks
NH = 2  # token halves per core (512 tokens each)
JH = 4  # 128-token chunks per half

f32 = mybir.dt.float32
f32r = mybir.dt.float32r
bf16 = mybir.dt.bfloat16

N_WARM = 5  # initial 512-wide dummy matmuls for the HAM clock gate

_CACHE: dict = {}


def _kernel_body(nc, tc, dram):
    x_re, wb_re, a16d, b16d, wg_re, exp_m, b_vec, out = dram

    from contextlib import ExitStack

    ctx = ExitStack()
    pw = ctx.enter_context(tc.tile_pool(name="weights", bufs=1))
    pg = ctx.enter_context(tc.tile_pool(name="gating", bufs=1))
    pmt = ctx.enter_context(tc.tile_pool(name="mmtmp", bufs=2))
    pout = ctx.enter_context(tc.tile_pool(name="outsb", bufs=3))
    ps_out = ctx.enter_context(tc.tile_pool(name="ps_out", bufs=4, space="PSUM"))
    ps_lgT = ctx.enter_context(tc.tile_pool(name="ps_lgT", bufs=1, space="PSUM"))
    ps_sm = ctx.enter_context(tc.tile_pool(name="ps_sm", bufs=1, space="PSUM"))
    ps_h = ctx.enter_context(tc.tile_pool(name="ps_h", bufs=1, space="PSUM"))
    ps_ce = ctx.enter_context(tc.tile_pool(name="ps_ce", bufs=1, space="PSUM"))

    # ---- PE prewarm: 512-wide garbage matmuls, never read ----------
    warm_sb = pw.tile([128, 512], f32r, tag="warm")
    nc.vector.memset(warm_sb.bitcast(f32), 0.0)
    n_warm = [0]

    def warm_mm():
        warm_ps = ps_out.tile(
            [128, 512], f32, tag="out", name=f"warm{n_warm[0]}"
        )
        nc.tensor.matmul(warm_ps, warm_sb[:, 0:128], warm_sb, start=True, stop=True)
        n_warm[0] += 1

    for _ in range(N_WARM):
        warm_mm()

    # identity for PE transposes: generate BEFORE any load dma_starts so
    # it doesn't queue behind them on its engine (v4 lesson: a 5us PE
    # stall + HAM re-throttle waiting for ident behind the load queue)
    ident = pw.tile([128, 128], f32, tag="ident")
    make_identity(nc, ident)

    # ---- Load phase: sync + gpsimd rings only, consumption order ----
    rings = [nc.sync, nc.gpsimd]

    wg_sb = pw.tile([128, DC, E], f32r, tag="wg")
    nc.gpsimd.dma_start(out=wg_sb, in_=wg_re)
    exp_sb = pw.tile([E, ER], f32r, tag="expand")
    nc.gpsimd.dma_start(out=exp_sb, in_=exp_m)

    # x half 0 (f32, contiguous [128,512] tiles), c-interleaved
    xt = [[None] * NH for _ in range(DC)]
    for c in range(DC):
        t = pw.tile([128, 512], f32r, tag=f"xt{c}_0")
        rings[c % 2].dma_start(out=t, in_=x_re[0, c])
        xt[c][0] = t

    # lora weights (bf16, small)
    a_sb = pw.tile([128, DC, ER], bf16, tag="a")
    nc.sync.dma_start(out=a_sb, in_=a16d.rearrange("p (c r) -> p c r", c=DC))
    b_sb = pw.tile([ER, D], bf16, tag="bflat")
    nc.gpsimd.dma_start(out=b_sb, in_=b16d)

    # W_base: one [128, 1024] bf16 tile per contraction chunk (both dt
    # slices arrive together, reused by both halves)
    wb = [None] * DC
    for c in range(DC):
        t = pw.tile([128, D], bf16, tag=f"wb{c}")
        rings[c % 2].dma_start(out=t, in_=wb_re[c])
        wb[c] = t

    # x half 1
    for c in range(DC):
        t = pw.tile([128, 512], f32r, tag=f"xt{c}_1")
        rings[c % 2].dma_start(out=t, in_=x_re[1, c])
        xt[c][1] = t

    bias_sb = pw.tile([128, D], f32, tag="bias")
    nc.gpsimd.dma_start(out=bias_sb, in_=b_vec.to_broadcast([128, D]))

    HT_sb = pg.tile([ER, NS], bf16, tag="HT")
    n_store = [0]

    out_r = out.rearrange("(hh j p) d -> hh p j d", p=128, j=JH)

    def store(out_sb, h, dsl):
        nc.scalar.dma_start(out=out_r[h, :, :, dsl], in_=out_sb)
        n_store[0] += 1

    # ---- Per-half pipeline ------------------------------------------
    xt16 = [[None] * NH for _ in range(DC)]

    for h in range(NH):
        hsl = slice(h * 512, (h + 1) * 512)

        # bf16 copies of this half's x chunks on the (load-free) Scalar
        # engine; the weight matmuls need uniform bf16 operands
        for c in range(DC):
            t = pw.tile([128, 512], bf16, tag=f"x16_{c}_{h}", name=f"x16_{c}_{h}")
            nc.scalar.activation(
                t, xt[c][h].bitcast(f32), mybir.ActivationFunctionType.Copy
            )
            xt16[c][h] = t

        # gating: logits^T accumulated over c (8-col stationary, cheap),
        # warm-fill matmuls keep PE duty high while x chunks arrive
        lgT_ps = ps_lgT.tile([E, 512], f32, tag="lgT", name=f"lgT{h}")
        for c in range(DC):
            nc.tensor.matmul(
                lgT_ps, wg_sb[:, c, :], xt[c][h], start=(c == 0), stop=(c == DC - 1)
            )
            if h == 0:
                warm_mm()
        lgT_sb = pg.tile([E, 512], f32, tag=f"lgTs{h}")
        nc.vector.tensor_copy(lgT_sb, lgT_ps)
        lg3_ps = ps_sm.tile([128, JH, E], f32, tag="sm", name=f"lg3{h}")
        for j in range(JH):
            nc.tensor.transpose(
                lg3_ps[:, j, :], lgT_sb[:, j * 128 : (j + 1) * 128], ident[0:E, 0:E]
            )
        lg_sb = pg.tile([128, JH, E], f32, tag=f"lg{h}")
        nc.vector.tensor_copy(lg_sb, lg3_ps)

        # lora up-projection: hT[(e,r), tok] over c chunks
        h_ps = ps_h.tile([ER, 512], f32, tag="h", name=f"hps{h}")
        for c in range(DC):
            nc.tensor.matmul(
                h_ps, a_sb[:, c, :], xt16[c][h], start=(c == 0), stop=(c == DC - 1)
            )
        h_sb = pmt.tile([ER, 512], f32, tag="hsb", name=f"hsb{h}")
        nc.vector.tensor_copy(h_sb, h_ps)

        # top-8 sort per token -> combine weights (DVE/ACT, overlaps PE)
        mx = pg.tile([128, JH, E], f32, tag=f"mx{h}")
        for j in range(JH):
            nc.vector.max(out=mx[:, j, :], in_=lg_sb[:, j, :])
        v1 = mx[:, :, 0:1]
        v2 = mx[:, :, 1:2]
        d21 = pg.tile([128, JH, 1], f32, tag=f"d21_{h}")
        nc.vector.tensor_sub(d21, v2, v1)
        w1 = pg.tile([128, JH, 1], f32, tag=f"w1_{h}")
        w2 = pg.tile([128, JH, 1], f32, tag=f"w2_{h}")
        nc.scalar.activation(w2, d21, mybir.ActivationFunctionType.Sigmoid)
        nc.scalar.activation(w1, d21, mybir.ActivationFunctionType.Sigmoid, scale=-1.0)

        eq1 = pg.tile([128, JH, E], f32, tag=f"eq1_{h}")
        eq2 = pg.tile([128, JH, E], f32, tag=f"eq2_{h}")
        cb = pg.tile([128, JH, E], f32, tag=f"cb{h}")
        bs = [128, JH, E]
        nc.vector.tensor_tensor(eq1, lg_sb, v1.to_broadcast(bs), mybir.AluOpType.is_equal)
        nc.vector.tensor_tensor(eq2, lg_sb, v2.to_broadcast(bs), mybir.AluOpType.is_equal)
        nc.vector.tensor_tensor(eq1, eq1, w1.to_broadcast(bs), mybir.AluOpType.mult)
        nc.vector.tensor_tensor(eq2, eq2, w2.to_broadcast(bs), mybir.AluOpType.mult)
        nc.vector.tensor_add(cb, eq1, eq2)

        # main accumulation dt0, c-streamed against W_base arrivals
        ops0 = [
            ps_out.tile([128, 512], f32, tag="out", name=f"ops{h}0{j}")
            for j in range(JH)
        ]
        for c in range(DC):
            for j in range(JH):
                jr = slice(j * 128, (j + 1) * 128)
                nc.tensor.matmul(
                    ops0[j], xt16[c][h][:, jr], wb[c][:, 0:512],
                    start=(c == 0), stop=False,
                )

        # combine^T via PE transpose, expand over ranks, mask h
        tp_ps = ps_sm.tile([E, JH, 128], f32, tag="sm", name=f"tp{h}")
        for j in range(JH):
            nc.tensor.transpose(tp_ps[:, j, :], cb[:, j, :], ident)
        cT_sb = pg.tile([E, 512], f32r, tag=f"cT{h}")
        nc.vector.tensor_copy(cT_sb, tp_ps)
        ce_ps = ps_ce.tile([ER, 512], f32, tag="ce", name=f"ce{h}")
        nc.tensor.matmul(ce_ps, exp_sb, cT_sb, start=True, stop=True)
        nc.vector.tensor_tensor(HT_sb[:, hsl], ce_ps, h_sb, mybir.AluOpType.mult)

        # H @ B into the dt0 psum banks, then drain
        for j in range(JH):
            gsl = slice(h * 512 + j * 128, h * 512 + (j + 1) * 128)
            nc.tensor.matmul(
                ops0[j], HT_sb[:, gsl], b_sb[:, 0:512], start=False, stop=True
            )
        ob0 = pout.tile([128, JH, 512], bf16, tag="osb", name=f"osb{h}0")
        for j in range(JH):
            nc.vector.tensor_add(ob0[:, j, :], ops0[j], bias_sb[:, 0:512])
        store(ob0, h, slice(0, 512))

        # dt1 (weights resident)
        ops1 = [
            ps_out.tile([128, 512], f32, tag="out", name=f"ops{h}1{j}")
            for j in range(JH)
        ]
        for c in range(DC):
            for j in range(JH):
                jr = slice(j * 128, (j + 1) * 128)
                nc.tensor.matmul(
                    ops1[j], xt16[c][h][:, jr], wb[c][:, 512:1024],
                    start=(c == 0), stop=False,
                )
        for j in range(JH):
            gsl = slice(h * 512 + j * 128, h * 512 + (j + 1) * 128)
            nc.tensor.matmul(
                ops1[j], HT_sb[:, gsl], b_sb[:, 512:1024], start=False, stop=True
            )
        if h < NH - 1:
            ob1 = pout.tile([128, JH, 512], bf16, tag="osb", name=f"osb{h}1")
            for j in range(JH):
                nc.vector.tensor_add(ob1[:, j, :], ops1[j], bias_sb[:, 512:1024])
            store(ob1, h, slice(512, 1024))
        else:
            # last group: per-j stores so the final store is small and
            # departs right after the last drain
            for j in range(JH):
                oj = pout.tile([128, 512], bf16, tag="osbj", name=f"osbj{j}")
                nc.vector.tensor_add(oj, ops1[j], bias_sb[:, 512:1024])
                nc.scalar.dma_start(
                    out=out[h * 512 + j * 128 : h * 512 + (j + 1) * 128, 512:1024],
                    in_=oj,
                )

    ctx.close()


def build_nc():
    nc = bacc.Bacc(
        "TRN2",
        target_bir_lowering=False,
        debug=False,
        enable_asserts=False,
        num_devices=CORES,
    )
    x_re = nc.dram_tensor("x_re", [NH, DC, 128, 512], f32, kind="ExternalInput").ap()
    wb_re = nc.dram_tensor("wb_re", [DC, 128, D], bf16, kind="ExternalInput").ap()
    a16d = nc.dram_tensor("a16", [128, DC * ER], bf16, kind="ExternalInput").ap()
    b16d = nc.dram_tensor("b16", [ER, D], bf16, kind="ExternalInput").ap()
    wg_re = nc.dram_tensor("wg_re", [128, DC, E], f32, kind="ExternalInput").ap()
    exp_m = nc.dram_tensor("exp_m", [E, ER], f32, kind="ExternalInput").ap()
    b_vec = nc.dram_tensor("b_vec", [1, D], f32, kind="ExternalInput").ap()
    out = nc.dram_tensor("out", [NS, D], bf16, kind="ExternalOutput").ap()

    dram = (
        x_re.bitcast(f32r),
        wb_re,
        a16d,
        b16d,
        wg_re.bitcast(f32r),
        exp_m.bitcast(f32r),
        b_vec,
        out,
    )
    with tile.TileContext(nc) as tc:
        _kernel_body(nc, tc, dram)
    nc.compile()
    return nc


def host_prep(x, W_gate, A, B, W_base, b_base):
    """Shard + lay out the full inputs into 8 per-core input maps.

    Every DMA tile is contiguous in DRAM:
      x_re  [NH, DC, 128, 512] f32 : x.T split into (half, chunk) tiles
      wb_re [DC, 128, D] bf16      : W_base.T row-chunks
      a16   [128, DC*ER] bf16      : A chunks, partition-major
      b16   [ER, D] bf16
      wg_re [128, DC, E] f32
    """
    xT = np.ascontiguousarray(x.T)  # [D, N]
    wb16 = np.ascontiguousarray(W_base.T).astype(ml_dtypes.bfloat16)
    wb_re = np.ascontiguousarray(wb16.reshape(DC, 128, D))
    a_fl = A.transpose(1, 0, 2).reshape(D, ER).astype(ml_dtypes.bfloat16)
    a16 = np.ascontiguousarray(
        a_fl.reshape(DC, 128, ER).transpose(1, 0, 2).reshape(128, DC * ER)
    )
    b16 = np.ascontiguousarray(B.reshape(ER, D).astype(ml_dtypes.bfloat16))
    wgT = np.ascontiguousarray(W_gate.T)  # [D, E]
    wg_re = np.ascontiguousarray(wgT.reshape(DC, 128, E).transpose(1, 0, 2))
    exp_m = np.zeros((E, ER), dtype=np.float32)
    for e in range(E):
        exp_m[e, e * R : (e + 1) * R] = 1.0
    b_vec = np.ascontiguousarray(b_base.reshape(1, D))

    in_maps = []
    for c in range(CORES):
        xc = xT[:, c * NS : (c + 1) * NS]  # [D, NS]
        x_re = np.ascontiguousarray(
            xc.reshape(DC, 128, NH, 512).transpose(2, 0, 1, 3)
        )
        in_maps.append(
            {
                "x_re": x_re,
                "wb_re": wb_re,
                "a16": a16,
                "b16": b16,
                "wg_re": wg_re,
                "exp_m": exp_m,
                "b_vec": b_vec,
            }
        )
    return in_maps


def kernel(x, W_gate, A, B, W_base, b_base):
    x = np.asarray(x, dtype=np.float32)
    W_gate = np.asarray(W_gate, dtype=np.float32)
    A = np.asarray(A, dtype=np.float32)
    B = np.asarray(B, dtype=np.float32)
    W_base = np.asarray(W_base, dtype=np.float32)
    b_base = np.asarray(b_base, dtype=np.float32)

    if "nc" not in _CACHE:
        _CACHE["nc"] = build_nc()
    nc = _CACHE["nc"]

    in_maps = host_prep(x, W_gate, A, B, W_base, b_base)
    res = run_bass_kernel_spmd(nc, in_maps, core_ids=list(range(CORES)))
    return np.concatenate(
        [res.results[c]["out"].astype(np.float32) for c in range(CORES)], axis=0
    )


# revision 13
# speedup vs baseline: 1.0818x; 1.0818x over previous
"""Trainium2 Bass kernel for MoE-with-LoRA-experts (nn_MoE_64098091925598).

Reference computation (N=8192 tokens, D=1024, E=8 experts, R=16, top-2):
    logits  = x @ W_gate.T                      [N, E]
    combine = scatter(softmax(top2(logits)))    [N, E] (2 nonzeros/row)
    moe     = sum_e combine[:,e] * (x @ A_e @ B_e)
    out     = moe + x @ W_base.T + b_base

Strategy: data-parallel over tokens across 8 NeuronCores (1024 tokens
per core); every core computes all 8 LoRA experts densely and masks by
the combine weights (H[n,(e,r)] = combine[n,e] * (x@A)[n,(e,r)], so the
weighted expert sum is one dense K=128 matmul H @ B_flat accumulated
into the same PSUM tile as the base linear).

v4 perf structure (vs v1's 76us):
- Loads are HBM-bound (~300 GB/s aggregate no matter how many DGE
  rings), so in-bytes are minimized: W_base/A/B are bf16 (host-cast),
  output is stored bf16 and upcast on host. x stays f32 so the top-2
  selection matches the fp32 reference; matmuls against bf16 weights
  use mixed-dtype operands (f32r x side, bf16 weight side) so no
  on-device conversion is needed.
- Loads run ONLY on the sync+gpsimd rings, in consumption order with
  contiguous host-prepped tiles. The Scalar engine issues no loads:
  its queue would head-of-line block on DGE ring backpressure and
  starve the sigmoid/combine chain (v2/v3 lesson). Scalar does the
  sigmoids and the output stores.
- HAM: the PE clock is throttled to 1.2 GHz until ~3.4us of sustained
  high-duty matmul activity. 512-wide garbage matmuls run before and
  between the DMA-paced gating matmuls to un-throttle early.
- Per 512-token half: gating (8 wide matmuls + 4 PE transposes), lora
  up-proj, dt0 base-accumulation c-streamed against W_base arrivals,
  combine-transpose + rank-expand, H@B into the same psum banks, then
  dt1 with everything resident. The DVE combine chain overlaps the
  base c-loop so the PE never waits on it.
"""

import numpy as np
import ml_dtypes

import concourse.mybir as mybir
import concourse.tile as tile
from concourse import bacc
from concourse.bass_utils import run_bass_kernel_spmd
from concourse.masks import make_identity

N_TOK, D, E, R, TOPK = 8192, 1024, 8, 16, 2
CORES = 8
NS = N_TOK // CORES  # tokens per core
ER = E * R  # 128, stacked expert-rank dim
DC = D // 128  # 8 contraction chunks
NH = 2  # token halves per core (512 tokens each)
JH = 4  # 128-token chunks per half

f32 = mybir.dt.float32
f32r = mybir.dt.float32r
bf16 = mybir.dt.bfloat16

N_WARM = 5  # initial 512-wide dummy matmuls for the HAM clock gate

# contraction-chunk consumption order matching the two-ring arrival
# interleave (sync: c0-3, gpsimd: c4-7)
C_ORD = [0, 4, 1, 5, 2, 6, 3, 7]

_CACHE: dict = {}


def _kernel_body(nc, tc, dram):
    x_re, wb_re, a16d, b16d, wg_re, exp_m, b_vec, out = dram

    from contextlib import ExitStack

    ctx = ExitStack()
    pw = ctx.enter_context(tc.tile_pool(name="weights", bufs=1))
    pg = ctx.enter_context(tc.tile_pool(name="gating", bufs=1))
    pmt = ctx.enter_context(tc.tile_pool(name="mmtmp", bufs=2))
    pout = ctx.enter_context(tc.tile_pool(name="outsb", bufs=3))
    ps_out = ctx.enter_context(tc.tile_pool(name="ps_out", bufs=4, space="PSUM"))
    ps_lgT = ctx.enter_context(tc.tile_pool(name="ps_lgT", bufs=1, space="PSUM"))
    ps_sm = ctx.enter_context(tc.tile_pool(name="ps_sm", bufs=1, space="PSUM"))
    ps_h = ctx.enter_context(tc.tile_pool(name="ps_h", bufs=1, space="PSUM"))
    ps_ce = ctx.enter_context(tc.tile_pool(name="ps_ce", bufs=1, space="PSUM"))

    # ---- PE prewarm: 512-wide garbage matmuls, never read ----------
    warm_sb = pw.tile([128, 512], f32r, tag="warm")
    nc.vector.memset(warm_sb.bitcast(f32), 0.0)
    n_warm = [0]

    def warm_mm():
        warm_ps = ps_out.tile(
            [128, 512], f32, tag="out", name=f"warm{n_warm[0]}"
        )
        nc.tensor.matmul(warm_ps, warm_sb[:, 0:128], warm_sb, start=True, stop=True)
        n_warm[0] += 1

    for _ in range(N_WARM):
        warm_mm()

    # identity for PE transposes: generate BEFORE any load dma_starts so
    # it doesn't queue behind them on its engine (v4 lesson: a 5us PE
    # stall + HAM re-throttle waiting for ident behind the load queue)
    ident = pw.tile([128, 128], f32, tag="ident")
    make_identity(nc, ident)

    # ---- Load phase: sync + gpsimd rings only, consumption order ----
    rings = [nc.sync, nc.gpsimd]

    wg_sb = pw.tile([128, DC, E], f32r, tag="wg")
    nc.sync.dma_start(out=wg_sb, in_=wg_re)
    exp_sb = pw.tile([E, ER], f32r, tag="expand")
    nc.gpsimd.dma_start(out=exp_sb, in_=exp_m)

    # x half 0 (f32, contiguous [128,512] tiles): sync carries c0-3,
    # gpsimd c4-7, so chunk availability alternates between ring heads
    xt = [[None] * NH for _ in range(DC)]
    for c in range(DC):
        t = pw.tile([128, 512], f32r, tag=f"xt{c}_0")
        rings[0 if c < 4 else 1].dma_start(out=t, in_=x_re[0, c])
        xt[c][0] = t

    # lora weights (bf16, small)
    a_sb = pw.tile([128, DC, ER], bf16, tag="a")
    nc.sync.dma_start(out=a_sb, in_=a16d.rearrange("p (c r) -> p c r", c=DC))
    b_sb = pw.tile([ER, D], bf16, tag="bflat")
    nc.gpsimd.dma_start(out=b_sb, in_=b16d)

    # W_base: one [128, 1024] bf16 tile per contraction chunk (both dt
    # slices arrive together, reused by both halves)
    wb = [None] * DC
    for c in range(DC):
        t = pw.tile([128, D], bf16, tag=f"wb{c}")
        rings[0 if c < 4 else 1].dma_start(out=t, in_=wb_re[c])
        wb[c] = t

    # x half 1
    for c in range(DC):
        t = pw.tile([128, 512], f32r, tag=f"xt{c}_1")
        rings[0 if c < 4 else 1].dma_start(out=t, in_=x_re[1, c])
        xt[c][1] = t

    bias_sb = pw.tile([128, D], f32, tag="bias")
    nc.gpsimd.dma_start(out=bias_sb, in_=b_vec.to_broadcast([128, D]))

    HT_sb = pg.tile([ER, NS], bf16, tag="HT")
    n_store = [0]

    out_r = out.rearrange("(hh j p) d -> hh p j d", p=128, j=JH)

    def store(out_sb, h, dsl):
        nc.scalar.dma_start(out=out_r[h, :, :, dsl], in_=out_sb)
        n_store[0] += 1

    # ---- Per-half pipeline ------------------------------------------
    xt16 = [[None] * NH for _ in range(DC)]

    for h in range(NH):
        hsl = slice(h * 512, (h + 1) * 512)

        # bf16 copies of this half's x chunks on DVE (NOT scalar: the
        # sigmoids behind them would delay the whole H@B chain)
        for c in C_ORD:
            t = pw.tile([128, 512], bf16, tag=f"x16_{c}_{h}", name=f"x16_{c}_{h}")
            nc.vector.tensor_copy(t, xt[c][h].bitcast(f32))
            xt16[c][h] = t

        # gating: logits^T accumulated over c (8-col stationary, cheap),
        # warm-fill matmuls keep PE duty high while x chunks arrive
        lgT_ps = ps_lgT.tile([E, 512], f32, tag="lgT", name=f"lgT{h}")
        for i, c in enumerate(C_ORD):
            nc.tensor.matmul(
                lgT_ps, wg_sb[:, c, :], xt[c][h], start=(i == 0), stop=(i == DC - 1)
            )
            if h == 0:
                warm_mm()
        lgT_sb = pg.tile([E, 512], f32, tag=f"lgTs{h}")
        nc.vector.tensor_copy(lgT_sb, lgT_ps)
        lg3_ps = ps_sm.tile([128, JH, E], f32, tag="sm", name=f"lg3{h}")
        for j in range(JH):
            nc.tensor.transpose(
                lg3_ps[:, j, :], lgT_sb[:, j * 128 : (j + 1) * 128], ident[0:E, 0:E]
            )
        lg_sb = pg.tile([128, JH, E], f32, tag=f"lg{h}")
        nc.vector.tensor_copy(lg_sb, lg3_ps)

        # lora up-projection: hT[(e,r), tok] over c chunks
        h_ps = ps_h.tile([ER, 512], f32, tag="h", name=f"hps{h}")
        for i, c in enumerate(C_ORD):
            nc.tensor.matmul(
                h_ps, a_sb[:, c, :], xt16[c][h], start=(i == 0), stop=(i == DC - 1)
            )
        h_sb = pmt.tile([ER, 512], f32, tag="hsb", name=f"hsb{h}")
        nc.vector.tensor_copy(h_sb, h_ps)

        # top-8 sort per token -> combine weights (DVE/ACT, overlaps PE)
        mx = pg.tile([128, JH, E], f32, tag=f"mx{h}")
        for j in range(JH):
            nc.vector.max(out=mx[:, j, :], in_=lg_sb[:, j, :])
        v1 = mx[:, :, 0:1]
        v2 = mx[:, :, 1:2]
        d21 = pg.tile([128, JH, 1], f32, tag=f"d21_{h}")
        nc.vector.tensor_sub(d21, v2, v1)
        w1 = pg.tile([128, JH, 1], f32, tag=f"w1_{h}")
        w2 = pg.tile([128, JH, 1], f32, tag=f"w2_{h}")
        nc.scalar.activation(w2, d21, mybir.ActivationFunctionType.Sigmoid)
        nc.scalar.activation(w1, d21, mybir.ActivationFunctionType.Sigmoid, scale=-1.0)

        eq1 = pg.tile([128, JH, E], f32, tag=f"eq1_{h}")
        eq2 = pg.tile([128, JH, E], f32, tag=f"eq2_{h}")
        cb = pg.tile([128, JH, E], f32, tag=f"cb{h}")
        bs = [128, JH, E]
        nc.vector.tensor_tensor(eq1, lg_sb, v1.to_broadcast(bs), mybir.AluOpType.is_equal)
        nc.vector.tensor_tensor(eq2, lg_sb, v2.to_broadcast(bs), mybir.AluOpType.is_equal)
        nc.vector.tensor_tensor(eq1, eq1, w1.to_broadcast(bs), mybir.AluOpType.mult)
        nc.vector.tensor_tensor(eq2, eq2, w2.to_broadcast(bs), mybir.AluOpType.mult)
        nc.vector.tensor_add(cb, eq1, eq2)

        # main accumulation dt0, c-streamed against W_base arrivals
        ops0 = [
            ps_out.tile([128, 512], f32, tag="out", name=f"ops{h}0{j}")
            for j in range(JH)
        ]
        for i, c in enumerate(C_ORD):
            for j in range(JH):
                jr = slice(j * 128, (j + 1) * 128)
                nc.tensor.matmul(
                    ops0[j], xt16[c][h][:, jr], wb[c][:, 0:512],
                    start=(i == 0), stop=False,
                )

        # combine^T via PE transpose, expand over ranks, mask h
        tp_ps = ps_sm.tile([E, JH, 128], f32, tag="sm", name=f"tp{h}")
        for j in range(JH):
            nc.tensor.transpose(tp_ps[:, j, :], cb[:, j, :], ident)
        cT_sb = pg.tile([E, 512], f32r, tag=f"cT{h}")
        nc.vector.tensor_copy(cT_sb, tp_ps)
        ce_ps = ps_ce.tile([ER, 512], f32, tag="ce", name=f"ce{h}")
        nc.tensor.matmul(ce_ps, exp_sb, cT_sb, start=True, stop=True)
        nc.vector.tensor_tensor(HT_sb[:, hsl], ce_ps, h_sb, mybir.AluOpType.mult)

        # H @ B into the dt0 psum banks, then drain
        for j in range(JH):
            gsl = slice(h * 512 + j * 128, h * 512 + (j + 1) * 128)
            nc.tensor.matmul(
                ops0[j], HT_sb[:, gsl], b_sb[:, 0:512], start=False, stop=True
            )
        ob0 = pout.tile([128, JH, 512], bf16, tag="osb", name=f"osb{h}0")
        for j in range(JH):
            nc.vector.tensor_add(ob0[:, j, :], ops0[j], bias_sb[:, 0:512])
        store(ob0, h, slice(0, 512))

        # dt1 (weights resident)
        ops1 = [
            ps_out.tile([128, 512], f32, tag="out", name=f"ops{h}1{j}")
            for j in range(JH)
        ]
        for i, c in enumerate(C_ORD):
            for j in range(JH):
                jr = slice(j * 128, (j + 1) * 128)
                nc.tensor.matmul(
                    ops1[j], xt16[c][h][:, jr], wb[c][:, 512:1024],
                    start=(i == 0), stop=False,
                )
        for j in range(JH):
            gsl = slice(h * 512 + j * 128, h * 512 + (j + 1) * 128)
            nc.tensor.matmul(
                ops1[j], HT_sb[:, gsl], b_sb[:, 512:1024], start=False, stop=True
            )
        if h < NH - 1:
            ob1 = pout.tile([128, JH, 512], bf16, tag="osb", name=f"osb{h}1")
            for j in range(JH):
                nc.vector.tensor_add(ob1[:, j, :], ops1[j], bias_sb[:, 512:1024])
            store(ob1, h, slice(512, 1024))
        else:
            # last group: per-j stores so the final store is small and
            # departs right after the last drain
            for j in range(JH):
                oj = pout.tile([128, 512], bf16, tag="osbj", name=f"osbj{j}")
                nc.vector.tensor_add(oj, ops1[j], bias_sb[:, 512:1024])
                nc.scalar.dma_start(
                    out=out[h * 512 + j * 128 : h * 512 + (j + 1) * 128, 512:1024],
                    in_=oj,
                )

    ctx.close()


def build_nc():
    nc = bacc.Bacc(
        "TRN2",
        target_bir_lowering=False,
        debug=False,
        enable_asserts=False,
        num_devices=CORES,
    )
    x_re = nc.dram_tensor("x_re", [NH, 2, DC // 2, 128, 512], f32, kind="ExternalInput").ap()
    wb_re = nc.dram_tensor("wb_re", [DC, 128, D], bf16, kind="ExternalInput").ap()
    a16d = nc.dram_tensor("a16", [128, DC * ER], bf16, kind="ExternalInput").ap()
    b16d = nc.dram_tensor("b16", [ER, D], bf16, kind="ExternalInput").ap()
    wg_re = nc.dram_tensor("wg_re", [128, DC, E], f32, kind="ExternalInput").ap()
    exp_m = nc.dram_tensor("exp_m", [E, ER], f32, kind="ExternalInput").ap()
    b_vec = nc.dram_tensor("b_vec", [1, D], f32, kind="ExternalInput").ap()
    out = nc.dram_tensor("out", [NS, D], bf16, kind="ExternalOutput").ap()

    dram = (
        x_re.bitcast(f32r),
        wb_re,
        a16d,
        b16d,
        wg_re.bitcast(f32r),
        exp_m.bitcast(f32r),
        b_vec,
        out,
    )
    with tile.TileContext(nc) as tc:
        _kernel_body(nc, tc, dram)
    nc.compile()
    return nc


def host_prep(x, W_gate, A, B, W_base, b_base):
    """Shard + lay out the full inputs into 8 per-core input maps.

    Every DMA tile is contiguous in DRAM:
      x_re  [NH, DC, 128, 512] f32 : x.T split into (half, chunk) tiles
      wb_re [DC, 128, D] bf16      : W_base.T row-chunks
      a16   [128, DC*ER] bf16      : A chunks, partition-major
      b16   [ER, D] bf16
      wg_re [128, DC, E] f32
    """
    xT = np.ascontiguousarray(x.T)  # [D, N]
    wb16 = np.ascontiguousarray(W_base.T).astype(ml_dtypes.bfloat16)
    wb_re = np.ascontiguousarray(wb16.reshape(DC, 128, D))
    a_fl = A.transpose(1, 0, 2).reshape(D, ER).astype(ml_dtypes.bfloat16)
    a16 = np.ascontiguousarray(
        a_fl.reshape(DC, 128, ER).transpose(1, 0, 2).reshape(128, DC * ER)
    )
    b16 = np.ascontiguousarray(B.reshape(ER, D).astype(ml_dtypes.bfloat16))
    wgT = np.ascontiguousarray(W_gate.T)  # [D, E]
    wg_re = np.ascontiguousarray(wgT.reshape(DC, 128, E).transpose(1, 0, 2))
    exp_m = np.zeros((E, ER), dtype=np.float32)
    for e in range(E):
        exp_m[e, e * R : (e + 1) * R] = 1.0
    b_vec = np.ascontiguousarray(b_base.reshape(1, D))

    in_maps = []
    for c in range(CORES):
        xc = xT[:, c * NS : (c + 1) * NS]  # [D, NS]
        x_re = np.ascontiguousarray(
            xc.reshape(DC, 128, NH, 512).transpose(2, 0, 1, 3)
        )
        in_maps.append(
            {
                "x_re": x_re,
                "wb_re": wb_re,
                "a16": a16,
                "b16": b16,
                "wg_re": wg_re,
                "exp_m": exp_m,
                "b_vec": b_vec,
            }
        )
    return in_maps


def kernel(x, W_gate, A, B, W_base, b_base):
    x = np.asarray(x, dtype=np.float32)
    W_gate = np.asarray(W_gate, dtype=np.float32)
    A = np.asarray(A, dtype=np.float32)
    B = np.asarray(B, dtype=np.float32)
    W_base = np.asarray(W_base, dtype=np.float32)
    b_base = np.asarray(b_base, dtype=np.float32)

    if "nc" not in _CACHE:
        _CACHE["nc"] = build_nc()
    nc = _CACHE["nc"]

    in_maps = host_prep(x, W_gate, A, B, W_base, b_base)
    res = run_bass_kernel_spmd(nc, in_maps, core_ids=list(range(CORES)))
    return np.concatenate(
        [res.results[c]["out"].astype(np.float32) for c in range(CORES)], axis=0
    )


# revision 15
# speedup vs baseline: 1.1197x; 1.0350x over previous
"""Trainium2 Bass kernel for MoE-with-LoRA-experts (nn_MoE_64098091925598).

Reference computation (N=8192 tokens, D=1024, E=8 experts, R=16, top-2):
    logits  = x @ W_gate.T                      [N, E]
    combine = scatter(softmax(top2(logits)))    [N, E] (2 nonzeros/row)
    moe     = sum_e combine[:,e] * (x @ A_e @ B_e)
    out     = moe + x @ W_base.T + b_base

Strategy: data-parallel over tokens across 8 NeuronCores (1024 tokens
per core); every core computes all 8 LoRA experts densely and masks by
the combine weights (H[n,(e,r)] = combine[n,e] * (x@A)[n,(e,r)], so the
weighted expert sum is one dense K=128 matmul H @ B_flat accumulated
into the same PSUM tile as the base linear).

v4 perf structure (vs v1's 76us):
- Loads are HBM-bound (~300 GB/s aggregate no matter how many DGE
  rings), so in-bytes are minimized: W_base/A/B are bf16 (host-cast),
  output is stored bf16 and upcast on host. x stays f32 so the top-2
  selection matches the fp32 reference; matmuls against bf16 weights
  use mixed-dtype operands (f32r x side, bf16 weight side) so no
  on-device conversion is needed.
- Loads run ONLY on the sync+gpsimd rings, in consumption order with
  contiguous host-prepped tiles. The Scalar engine issues no loads:
  its queue would head-of-line block on DGE ring backpressure and
  starve the sigmoid/combine chain (v2/v3 lesson). Scalar does the
  sigmoids and the output stores.
- HAM: the PE clock is throttled to 1.2 GHz until ~3.4us of sustained
  high-duty matmul activity. 512-wide garbage matmuls run before and
  between the DMA-paced gating matmuls to un-throttle early.
- Per 512-token half: gating (8 wide matmuls + 4 PE transposes), lora
  up-proj, dt0 base-accumulation c-streamed against W_base arrivals,
  combine-transpose + rank-expand, H@B into the same psum banks, then
  dt1 with everything resident. The DVE combine chain overlaps the
  base c-loop so the PE never waits on it.
"""

import numpy as np
import ml_dtypes

import concourse.mybir as mybir
import concourse.tile as tile
from concourse import bacc
from concourse.bass_utils import run_bass_kernel_spmd
from concourse.masks import make_identity

N_TOK, D, E, R, TOPK = 8192, 1024, 8, 16, 2
CORES = 8
NS = N_TOK // CORES  # tokens per core
ER = E * R  # 128, stacked expert-rank dim
DC = D // 128  # 8 contraction chunks
NH = 2  # token halves per core (512 tokens each)
JH = 4  # 128-token chunks per half

f32 = mybir.dt.float32
f32r = mybir.dt.float32r
bf16 = mybir.dt.bfloat16

N_WARM = 5  # initial 512-wide dummy matmuls for the HAM clock gate

# contraction-chunk consumption order matching the two-ring arrival
# interleave (sync: c0-3, gpsimd: c4-7)
C_ORD = [0, 4, 1, 5, 2, 6, 3, 7]

_CACHE: dict = {}


def _kernel_body(nc, tc, dram):
    x_re, wb_re, a16d, b16d, wg_re, exp_m, b_vec, out = dram

    from contextlib import ExitStack

    ctx = ExitStack()
    pw = ctx.enter_context(tc.tile_pool(name="weights", bufs=1))
    pg = ctx.enter_context(tc.tile_pool(name="gating", bufs=1))
    pmt = ctx.enter_context(tc.tile_pool(name="mmtmp", bufs=2))
    pout = ctx.enter_context(tc.tile_pool(name="outsb", bufs=3))
    ps_out = ctx.enter_context(tc.tile_pool(name="ps_out", bufs=4, space="PSUM"))
    ps_lgT = ctx.enter_context(tc.tile_pool(name="ps_lgT", bufs=1, space="PSUM"))
    ps_sm = ctx.enter_context(tc.tile_pool(name="ps_sm", bufs=1, space="PSUM"))
    ps_h = ctx.enter_context(tc.tile_pool(name="ps_h", bufs=1, space="PSUM"))
    ps_ce = ctx.enter_context(tc.tile_pool(name="ps_ce", bufs=1, space="PSUM"))

    # ---- PE prewarm: 512-wide garbage matmuls, never read ----------
    warm_sb = pw.tile([128, 512], f32r, tag="warm")
    nc.vector.memset(warm_sb.bitcast(f32), 0.0)
    n_warm = [0]

    def warm_mm():
        warm_ps = ps_out.tile(
            [128, 512], f32, tag="out", name=f"warm{n_warm[0]}"
        )
        nc.tensor.matmul(warm_ps, warm_sb[:, 0:128], warm_sb, start=True, stop=True)
        n_warm[0] += 1

    for _ in range(N_WARM):
        warm_mm()

    # identity for PE transposes: generate BEFORE any load dma_starts so
    # it doesn't queue behind them on its engine (v4 lesson: a 5us PE
    # stall + HAM re-throttle waiting for ident behind the load queue)
    ident = pw.tile([128, 128], f32, tag="ident")
    make_identity(nc, ident)

    # ---- Load phase: sync + gpsimd rings only, consumption order ----
    rings = [nc.sync, nc.gpsimd]

    wg_sb = pw.tile([128, DC, E], f32r, tag="wg")
    nc.sync.dma_start(out=wg_sb, in_=wg_re)
    exp_sb = pw.tile([E, ER], f32r, tag="expand")
    nc.gpsimd.dma_start(out=exp_sb, in_=exp_m)

    # Loads are merged into ONE dma per ring per phase: per-dma_start
    # fixed cost (~2us descriptor-gen + completion latency) dominated
    # the load wall with 18 small dmas. x: sync carries c0-3, gpsimd
    # c4-7; W_base row-chunks likewise; both dt slices of W_base arrive
    # together and are reused by both halves.
    xt = [[None] * NH for _ in range(DC)]
    xh00 = pw.tile([128, DC // 2, 512], f32r, tag="xh00")
    nc.sync.dma_start(out=xh00, in_=x_re[0, 0].rearrange("c p n -> p c n"))
    xh01 = pw.tile([128, DC // 2, 512], f32r, tag="xh01")
    nc.gpsimd.dma_start(out=xh01, in_=x_re[0, 1].rearrange("c p n -> p c n"))
    for c in range(DC):
        xt[c][0] = (xh00 if c < 4 else xh01)[:, c % 4, :]

    # lora weights (bf16, small)
    a_sb = pw.tile([128, DC, ER], bf16, tag="a")
    nc.sync.dma_start(out=a_sb, in_=a16d.rearrange("p (c r) -> p c r", c=DC))
    b_sb = pw.tile([ER, D], bf16, tag="bflat")
    nc.gpsimd.dma_start(out=b_sb, in_=b16d)

    # W_base
    wbA = pw.tile([128, DC // 2, D], bf16, tag="wbA")
    nc.sync.dma_start(out=wbA, in_=wb_re[0:4].rearrange("c p d -> p c d"))
    wbB = pw.tile([128, DC // 2, D], bf16, tag="wbB")
    nc.gpsimd.dma_start(out=wbB, in_=wb_re[4:8].rearrange("c p d -> p c d"))
    wb = [(wbA if c < 4 else wbB)[:, c % 4, :] for c in range(DC)]

    # x half 1
    xh10 = pw.tile([128, DC // 2, 512], f32r, tag="xh10")
    nc.sync.dma_start(out=xh10, in_=x_re[1, 0].rearrange("c p n -> p c n"))
    xh11 = pw.tile([128, DC // 2, 512], f32r, tag="xh11")
    nc.gpsimd.dma_start(out=xh11, in_=x_re[1, 1].rearrange("c p n -> p c n"))
    for c in range(DC):
        xt[c][1] = (xh10 if c < 4 else xh11)[:, c % 4, :]

    bias_sb = pw.tile([128, D], f32, tag="bias")
    nc.gpsimd.dma_start(out=bias_sb, in_=b_vec.to_broadcast([128, D]))

    HT_sb = pg.tile([ER, NS], bf16, tag="HT")
    n_store = [0]

    out_r = out.rearrange("(hh j p) d -> hh p j d", p=128, j=JH)

    def store(out_sb, h, dsl):
        nc.scalar.dma_start(out=out_r[h, :, :, dsl], in_=out_sb)
        n_store[0] += 1

    # ---- Per-half pipeline ------------------------------------------
    xt16 = [[None] * NH for _ in range(DC)]

    for h in range(NH):
        hsl = slice(h * 512, (h + 1) * 512)

        # bf16 copies of this half's x chunks on DVE (NOT scalar: the
        # sigmoids behind them would delay the whole H@B chain)
        for c in C_ORD:
            t = pw.tile([128, 512], bf16, tag=f"x16_{c}_{h}", name=f"x16_{c}_{h}")
            nc.vector.tensor_copy(t, xt[c][h].bitcast(f32))
            xt16[c][h] = t

        # gating: logits^T accumulated over c (8-col stationary, cheap),
        # warm-fill matmuls keep PE duty high while x chunks arrive
        lgT_ps = ps_lgT.tile([E, 512], f32, tag="lgT", name=f"lgT{h}")
        for i, c in enumerate(C_ORD):
            nc.tensor.matmul(
                lgT_ps, wg_sb[:, c, :], xt[c][h], start=(i == 0), stop=(i == DC - 1)
            )
            if h == 0:
                warm_mm()
        lgT_sb = pg.tile([E, 512], f32, tag=f"lgTs{h}")
        nc.vector.tensor_copy(lgT_sb, lgT_ps)
        lg3_ps = ps_sm.tile([128, JH, E], f32, tag="sm", name=f"lg3{h}")
        for j in range(JH):
            nc.tensor.transpose(
                lg3_ps[:, j, :], lgT_sb[:, j * 128 : (j + 1) * 128], ident[0:E, 0:E]
            )
        lg_sb = pg.tile([128, JH, E], f32, tag=f"lg{h}")
        nc.vector.tensor_copy(lg_sb, lg3_ps)

        # lora up-projection: hT[(e,r), tok] over c chunks
        h_ps = ps_h.tile([ER, 512], f32, tag="h", name=f"hps{h}")
        for i, c in enumerate(C_ORD):
            nc.tensor.matmul(
                h_ps, a_sb[:, c, :], xt16[c][h], start=(i == 0), stop=(i == DC - 1)
            )
        h_sb = pmt.tile([ER, 512], f32, tag="hsb", name=f"hsb{h}")
        nc.vector.tensor_copy(h_sb, h_ps)

        # top-8 sort per token -> combine weights (DVE/ACT, overlaps PE)
        mx = pg.tile([128, JH, E], f32, tag=f"mx{h}")
        for j in range(JH):
            nc.vector.max(out=mx[:, j, :], in_=lg_sb[:, j, :])
        v1 = mx[:, :, 0:1]
        v2 = mx[:, :, 1:2]
        d21 = pg.tile([128, JH, 1], f32, tag=f"d21_{h}")
        nc.vector.tensor_sub(d21, v2, v1)
        w1 = pg.tile([128, JH, 1], f32, tag=f"w1_{h}")
        w2 = pg.tile([128, JH, 1], f32, tag=f"w2_{h}")
        nc.scalar.activation(w2, d21, mybir.ActivationFunctionType.Sigmoid)
        nc.scalar.activation(w1, d21, mybir.ActivationFunctionType.Sigmoid, scale=-1.0)

        eq1 = pg.tile([128, JH, E], f32, tag=f"eq1_{h}")
        eq2 = pg.tile([128, JH, E], f32, tag=f"eq2_{h}")
        cb = pg.tile([128, JH, E], f32, tag=f"cb{h}")
        bs = [128, JH, E]
        nc.vector.tensor_tensor(eq1, lg_sb, v1.to_broadcast(bs), mybir.AluOpType.is_equal)
        nc.vector.tensor_tensor(eq2, lg_sb, v2.to_broadcast(bs), mybir.AluOpType.is_equal)
        nc.vector.tensor_tensor(eq1, eq1, w1.to_broadcast(bs), mybir.AluOpType.mult)
        nc.vector.tensor_tensor(eq2, eq2, w2.to_broadcast(bs), mybir.AluOpType.mult)
        nc.vector.tensor_add(cb, eq1, eq2)

        # main accumulation dt0, c-streamed against W_base arrivals
        ops0 = [
            ps_out.tile([128, 512], f32, tag="out", name=f"ops{h}0{j}")
            for j in range(JH)
        ]
        for i, c in enumerate(C_ORD):
            for j in range(JH):
                jr = slice(j * 128, (j + 1) * 128)
                nc.tensor.matmul(
                    ops0[j], xt16[c][h][:, jr], wb[c][:, 0:512],
                    start=(i == 0), stop=False,
                )

        # combine^T via PE transpose, expand over ranks, mask h
        tp_ps = ps_sm.tile([E, JH, 128], f32, tag="sm", name=f"tp{h}")
        for j in range(JH):
            nc.tensor.transpose(tp_ps[:, j, :], cb[:, j, :], ident)
        cT_sb = pg.tile([E, 512], f32r, tag=f"cT{h}")
        nc.vector.tensor_copy(cT_sb, tp_ps)
        ce_ps = ps_ce.tile([ER, 512], f32, tag="ce", name=f"ce{h}")
        nc.tensor.matmul(ce_ps, exp_sb, cT_sb, start=True, stop=True)
        nc.vector.tensor_tensor(HT_sb[:, hsl], ce_ps, h_sb, mybir.AluOpType.mult)

        # H @ B into the dt0 psum banks, then drain
        for j in range(JH):
            gsl = slice(h * 512 + j * 128, h * 512 + (j + 1) * 128)
            nc.tensor.matmul(
                ops0[j], HT_sb[:, gsl], b_sb[:, 0:512], start=False, stop=True
            )
        ob0 = pout.tile([128, JH, 512], bf16, tag="osb", name=f"osb{h}0")
        for j in range(JH):
            nc.vector.tensor_add(ob0[:, j, :], ops0[j], bias_sb[:, 0:512])
        store(ob0, h, slice(0, 512))

        # dt1 (weights resident)
        ops1 = [
            ps_out.tile([128, 512], f32, tag="out", name=f"ops{h}1{j}")
            for j in range(JH)
        ]
        for i, c in enumerate(C_ORD):
            for j in range(JH):
                jr = slice(j * 128, (j + 1) * 128)
                nc.tensor.matmul(
                    ops1[j], xt16[c][h][:, jr], wb[c][:, 512:1024],
                    start=(i == 0), stop=False,
                )
        for j in range(JH):
            gsl = slice(h * 512 + j * 128, h * 512 + (j + 1) * 128)
            nc.tensor.matmul(
                ops1[j], HT_sb[:, gsl], b_sb[:, 512:1024], start=False, stop=True
            )
        if h < NH - 1:
            ob1 = pout.tile([128, JH, 512], bf16, tag="osb", name=f"osb{h}1")
            for j in range(JH):
                nc.vector.tensor_add(ob1[:, j, :], ops1[j], bias_sb[:, 512:1024])
            store(ob1, h, slice(512, 1024))
        else:
            # last group: per-j stores so the final store is small and
            # departs right after the last drain
            for j in range(JH):
                oj = pout.tile([128, 512], bf16, tag="osbj", name=f"osbj{j}")
                nc.vector.tensor_add(oj, ops1[j], bias_sb[:, 512:1024])
                nc.scalar.dma_start(
                    out=out[h * 512 + j * 128 : h * 512 + (j + 1) * 128, 512:1024],
                    in_=oj,
                )

    ctx.close()


def build_nc():
    nc = bacc.Bacc(
        "TRN2",
        target_bir_lowering=False,
        debug=False,
        enable_asserts=False,
        num_devices=CORES,
    )
    x_re = nc.dram_tensor("x_re", [NH, 2, DC // 2, 128, 512], f32, kind="ExternalInput").ap()
    wb_re = nc.dram_tensor("wb_re", [DC, 128, D], bf16, kind="ExternalInput").ap()
    a16d = nc.dram_tensor("a16", [128, DC * ER], bf16, kind="ExternalInput").ap()
    b16d = nc.dram_tensor("b16", [ER, D], bf16, kind="ExternalInput").ap()
    wg_re = nc.dram_tensor("wg_re", [128, DC, E], f32, kind="ExternalInput").ap()
    exp_m = nc.dram_tensor("exp_m", [E, ER], f32, kind="ExternalInput").ap()
    b_vec = nc.dram_tensor("b_vec", [1, D], f32, kind="ExternalInput").ap()
    out = nc.dram_tensor("out", [NS, D], bf16, kind="ExternalOutput").ap()

    dram = (
        x_re.bitcast(f32r),
        wb_re,
        a16d,
        b16d,
        wg_re.bitcast(f32r),
        exp_m.bitcast(f32r),
        b_vec,
        out,
    )
    with tile.TileContext(nc) as tc:
        _kernel_body(nc, tc, dram)
    nc.compile()
    return nc


def host_prep(x, W_gate, A, B, W_base, b_base):
    """Shard + lay out the full inputs into 8 per-core input maps.

    Every DMA tile is contiguous in DRAM:
      x_re  [NH, DC, 128, 512] f32 : x.T split into (half, chunk) tiles
      wb_re [DC, 128, D] bf16      : W_base.T row-chunks
      a16   [128, DC*ER] bf16      : A chunks, partition-major
      b16   [ER, D] bf16
      wg_re [128, DC, E] f32
    """
    xT = np.ascontiguousarray(x.T)  # [D, N]
    wb16 = np.ascontiguousarray(W_base.T).astype(ml_dtypes.bfloat16)
    wb_re = np.ascontiguousarray(wb16.reshape(DC, 128, D))
    a_fl = A.transpose(1, 0, 2).reshape(D, ER).astype(ml_dtypes.bfloat16)
    a16 = np.ascontiguousarray(
        a_fl.reshape(DC, 128, ER).transpose(1, 0, 2).reshape(128, DC * ER)
    )
    b16 = np.ascontiguousarray(B.reshape(ER, D).astype(ml_dtypes.bfloat16))
    wgT = np.ascontiguousarray(W_gate.T)  # [D, E]
    wg_re = np.ascontiguousarray(wgT.reshape(DC, 128, E).transpose(1, 0, 2))
    exp_m = np.zeros((E, ER), dtype=np.float32)
    for e in range(E):
        exp_m[e, e * R : (e + 1) * R] = 1.0
    b_vec = np.ascontiguousarray(b_base.reshape(1, D))

    in_maps = []
    for c in range(CORES):
        xc = xT[:, c * NS : (c + 1) * NS]  # [D, NS]
        x_re = np.ascontiguousarray(
            xc.reshape(DC, 128, NH, 512).transpose(2, 0, 1, 3)
        ).reshape(NH, 2, DC // 2, 128, 512)
        in_maps.append(
            {
                "x_re": x_re,
                "wb_re": wb_re,
                "a16": a16,
                "b16": b16,
                "wg_re": wg_re,
                "exp_m": exp_m,
                "b_vec": b_vec,
            }
        )
    return in_maps


def kernel(x, W_gate, A, B, W_base, b_base):
    x = np.asarray(x, dtype=np.float32)
    W_gate = np.asarray(W_gate, dtype=np.float32)
    A = np.asarray(A, dtype=np.float32)
    B = np.asarray(B, dtype=np.float32)
    W_base = np.asarray(W_base, dtype=np.float32)
    b_base = np.asarray(b_base, dtype=np.float32)

    if "nc" not in _CACHE:
        _CACHE["nc"] = build_nc()
    nc = _CACHE["nc"]

    in_maps = host_prep(x, W_gate, A, B, W_base, b_base)
    res = run_bass_kernel_spmd(nc, in_maps, core_ids=list(range(CORES)))
    return np.concatenate(
        [res.results[c]["out"].astype(np.float32) for c in range(CORES)], axis=0
    )
